# revision 5
# baseline (speedup 1.0000x reference)
"""Multi-head attention kernel for Trainium2 (8 NeuronCores, SPMD).

Problem: x [4,1,2048,3], W_query/W_key/W_value [1,8,3,3] ->
ctx [4,8,2048,3] = softmax((x Wq)(x Wk)^T / sqrt(3)) @ (x Wv), returned
as a (ctx, ctx) tuple matching the reference.

Sharding: 32 (batch, head) blocks over 8 cores -> core c owns batch c//2,
heads 4*(c%2) .. +4. Each core runs an identical Bass program on its slice.

Per-core device program (S=2048, heads processed in 2 pairs):
  - ACT (exp) is the roofline: 4*2048*2048 elements at 1 elem/lane/cycle
    @1.2 GHz ~= 109us + ~260 cycles/instruction overhead. Everything is
    organized to keep ACT ~100% busy on the largest exp tiles PSUM allows
    and to keep every other engine off its critical path.
  - Host precomputes Q/K projections and their 3-way bf16 splits directly
    in the stacked [128, 2048] device layout (6 product terms x 3 dims =
    18 rows per 32-row head group), so there is no on-device setup phase.
  - PSUM: s pool = 2 x [128, 1536] f32 (banks 0-5, double buffered);
    ctx = one persistent [128, 1024] tile (banks 6-7) per chunk.
  - Per (pair, query-chunk) = 32 units of [128 keys, 512 queries], packed
    3 units per s buffer (heads interleaved kt-major):
      PE:  per unit one QK matmul [32,128]x[32,512] in the head's 32-row
           group (row tiling; concurrent units land in different banks)
      ACT: one exp per buffer (F=1536/1024) -> bf16 P tiles in SBUF
      PE:  per key tile two PV matmuls [128,7]x[128,512], both in col
           group 0 (serialized) -> ctx banks 6/7 at partitions 0:7.
           Full-128-row PV duty is what keeps the PE's activity monitor
           warm (2.4 GHz); concurrent col-tiled PVs halve that duty and
           the clock collapses to 0.65 GHz (measured). Concurrent
           matmuls must also never share a PSUM bank (same-bank
           concurrent writes corrupt each other - verified on HW).
           PSUM start/stop accumulation over the 16 key tiles does the
           reduction for free.
  - Normalize (dripped one piece per buffer during the next chunk): one
    DVE copy moves both heads' ctx [7+denom rows, 512] to SBUF; fused
    transpose+Wv matmuls (row groups 0/1 -> s-tile banks 1/2) produce
    [q, 3e+denom] blocks; DVE reciprocal + per-partition scalar multiply;
    DMA out. ct blocks alias into the just-consumed s buffer, so they
    need no PSUM of their own.
"""

import math

import numpy as np
import ml_dtypes

import concourse.bass as bass
import concourse.bacc as bacc
import concourse.tile as tile
from concourse import mybir
from concourse.bass_utils import run_bass_kernel_spmd

f32 = mybir.dt.float32
f32r = mybir.dt.float32r
bf16 = mybir.dt.bfloat16
i16 = mybir.dt.int16
EXP = mybir.ActivationFunctionType.Exp

B, H, S, D = 4, 8, 2048, 3
NCORES = 8
HPC = H // 2           # heads per core = 4
QCH = 512              # query chunk
NQ = S // QCH          # 4
KT = 128               # key tile
NKT = S // KT          # 16
NC4 = QCH // KT        # 128-query blocks per chunk = 4
NU = 2 * NKT           # units per (pair, chunk) = 32
NBUF = (NU + 2) // 3   # s buffers per chunk = 11 (10x3 + 1x2)
SCALE = 1.0 / math.sqrt(D)

# DVE-exp (Schraudolph on bf16 bits): for buffers in DVE_BUFS the exp is a
# single DVE tensor_scalar: i16 = round(s*EXPC1 + EXPC2); bitcast bf16 is
# ~exp(s*SCALE) with <=3.3% per-element error (HW-verified round-to-nearest).
# Splitting the exp work ACT/DVE nearly doubles softmax throughput; end-to-end
# rel err simulated at 4.1e-3 vs the 2e-2 gate.
EXPC1 = SCALE * 128.0 / math.log(2.0)
EXPC2 = 127.0 * 128.0 - 5.568
DVE_BUFS = frozenset((1, 4, 7, 9))

# 3-way bf16 split product terms kept for q.k (drop (2,3),(3,2),(3,3))
Q_ORDER = (0, 0, 1, 0, 2, 1)
K_ORDER = (0, 1, 0, 2, 0, 1)


def _split_hi_lo(a: np.ndarray):
    """Exact split a = hi + lo with both parts f32r-representable
    (11-bit mantissa, round-to-nearest with carry)."""
    a = np.ascontiguousarray(a, dtype=np.float32)
    u = a.view(np.uint32)
    r = (u + np.uint32(0x7FF) + ((u >> np.uint32(12)) & np.uint32(1))) & np.uint32(
        0xFFFFF000
    )
    hi = r.view(np.float32)
    lo = (a - hi).astype(np.float32)
    return hi, lo


def _split3_bf16(a: np.ndarray):
    """3-way bf16 split: a ~= a1 + a2 + a3, each bf16."""
    a = np.ascontiguousarray(a, dtype=np.float32)
    a1 = a.astype(ml_dtypes.bfloat16)
    r = a - a1.astype(np.float32)
    a2 = r.astype(ml_dtypes.bfloat16)
    a3 = (r - a2.astype(np.float32)).astype(ml_dtypes.bfloat16)
    return a1, a2, a3


def _build_nc():
    nc = bacc.Bacc("TRN2", target_bir_lowering=False, debug=False,
                   num_devices=NCORES)

    qstk_in = nc.dram_tensor("qstk", [128, S], bf16, kind="ExternalInput").ap()
    kstk_in = nc.dram_tensor("kstk", [128, S], bf16, kind="ExternalInput").ap()
    xo_in = nc.dram_tensor("xo", [128, NKT, 7], bf16, kind="ExternalInput").ap()
    wv7_in = nc.dram_tensor("wv7", [128, 16], bf16, kind="ExternalInput").ap()
    out = nc.dram_tensor("out", [HPC, S, D], f32, kind="ExternalOutput").ap()

    with tile.TileContext(nc) as tc:
        with tc.tile_pool(name="per", bufs=1) as per, \
             tc.tile_pool(name="work", bufs=1) as work, \
             tc.tile_pool(name="spool", bufs=2, space="PSUM") as spool, \
             tc.tile_pool(name="cpool", bufs=1, space="PSUM") as cpool:
            qstk = per.tile([128, S], bf16)
            kstk = per.tile([128, S], bf16)
            xo = per.tile([128, NKT, 7], bf16)
            wv7 = per.tile([128, 16], bf16)

            # ACT exp-table preload: a 1-element exp with no upstream deps
            # makes the ~2.7us ACT_TABLE_LOAD overlap the input DMAs.
            tdum = per.tile([128, 1], f32)
            tdum2 = per.tile([128, 1], f32)
            nc.gpsimd.memset(tdum, 0.0)
            nc.scalar.activation(tdum2, tdum, EXP)

            # pair-0 head rows first so the first QK can start ~1.5us in;
            # the rest streams behind it on both queues
            nc.sync.dma_start(out=kstk[0:64, :], in_=kstk_in[0:64, :])
            nc.sync.dma_start(out=qstk[0:64, 0:QCH], in_=qstk_in[0:64, 0:QCH])
            nc.gpsimd.dma_start(out=wv7, in_=wv7_in)
            nc.gpsimd.dma_start(out=xo, in_=xo_in)
            nc.sync.dma_start(out=qstk[0:64, QCH:], in_=qstk_in[0:64, QCH:])
            nc.gpsimd.dma_start(out=kstk[64:128, :], in_=kstk_in[64:128, :])
            nc.sync.dma_start(out=qstk[64:128, :], in_=qstk_in[64:128, :])

            pending = []  # deferred normalize pieces; each takes the
                          # current post-exp score buffer
            rec_ctr = [0]

            def emit_buffer(p, qc, b):
                """QK matmuls for s-buffer b of chunk (p, qc)."""
                s = spool.tile([128, 3 * QCH], f32, name=f"s{p}{qc}_{b}",
                               tag="s")
                for u in range(3 * b, min(3 * b + 3, NU)):
                    t, hl = u // 2, u % 2
                    h = 2 * p + hl
                    nc.tensor.matmul(
                        s[:, (u % 3) * QCH:(u % 3 + 1) * QCH],
                        lhsT=kstk[32 * h:32 * h + 32, t * KT:(t + 1) * KT],
                        rhs=qstk[32 * h:32 * h + 32, qc * QCH:(qc + 1) * QCH],
                        start=True, stop=True,
                        tile_position=(32 * h, 0),
                    )
                return s

            def mk_copy(_ctx, _sb):
                def go(s_exp):
                    nc.vector.tensor_copy(
                        _sb[0:7, 0:2 * QCH], _ctx[0:7, 0:2 * QCH])
                return go

            def mk_group(c4, p, _sb, _ost):
                # fused transpose + Wv contraction + denom for one
                # 128-query block, both heads of the pair, plus the
                # normalization divides. ct rows land in banks 1 (even
                # head) and 2 (odd head) of the current score buffer,
                # which the exp has already consumed; the next-next QK
                # rewrite of that buffer is emitted after these readers.
                def go(s_exp):
                    for hl in range(2):
                        base = (1 + hl) * QCH + 16 * c4
                        nc.tensor.matmul(
                            s_exp[:, base:base + 4],
                            lhsT=_sb[0:7,
                                     hl * QCH + c4 * KT:
                                     hl * QCH + (c4 + 1) * KT],
                            rhs=wv7[0:7, 8 * p + 4 * hl:8 * p + 4 * hl + 4],
                            start=True, stop=True,
                            tile_position=(0, 0),
                        )
                    rec_ctr[0] += 1
                    rec = work.tile([128, 2], f32, name=f"r{rec_ctr[0]}",
                                    tag="rec", bufs=3)
                    nc.vector.reciprocal(
                        rec,
                        s_exp[:, QCH + 16 * c4 + 3:2 * QCH + 16 * c4 + 4:QCH])
                    for hl in range(2):
                        base = (1 + hl) * QCH + 16 * c4
                        nc.vector.tensor_scalar_mul(
                            _ost[:, c4, hl, :],
                            s_exp[:, base:base + 3],
                            rec[:, hl:hl + 1])
                go.needs_s = True
                return go

            def mk_out(p, qc, hl, _ost):
                def go(s_exp):
                    h = 2 * p + hl
                    dst = bass.AP(
                        tensor=out.tensor,
                        offset=(h * S * D + qc * QCH * D),
                        ap=[[D, 128], [KT * D, NC4], [1, D]],
                    )
                    nc.sync.dma_start(out=dst, in_=_ost[:, :, hl, :])
                return go

            s_cur = emit_buffer(0, 0, 0)
            chunks = [(p, qc) for p in range(2) for qc in range(NQ)]
            for ci, (p, qc) in enumerate(chunks):
                ctx = cpool.tile([128, 2 * QCH], f32, name=f"ctx{p}{qc}",
                                 tag="ctx")
                ctxsb = work.tile([128, 2 * QCH], bf16, name=f"cs{p}{qc}",
                                  tag="ctxsb", bufs=2)
                ostage = work.tile([128, NC4, 2, D], f32, name=f"ost{p}{qc}",
                                   tag="ost", bufs=2)
                ptiles = {}
                done_kt = 0
                for b in range(NBUF):
                    nu = min(3 * b + 3, NU) - 3 * b
                    fsz = nu * QCH
                    if b in DVE_BUFS:
                        pt = work.tile([128, 3 * QCH], i16,
                                       name=f"pi{p}{qc}_{b}", tag="pi", bufs=3)
                        ptiles[b] = (pt, True)
                        nc.vector.tensor_scalar(
                            out=pt[:, 0:fsz], in0=s_cur[:, 0:fsz],
                            scalar1=EXPC1, scalar2=EXPC2,
                            op0=mybir.AluOpType.mult, op1=mybir.AluOpType.add)
                    else:
                        pt = work.tile([128, 3 * QCH], bf16,
                                       name=f"p{p}{qc}_{b}", tag="p", bufs=3)
                        ptiles[b] = (pt, False)
                        nc.scalar.activation(pt[:, 0:fsz], s_cur[:, 0:fsz],
                                             EXP, scale=SCALE)
                    s_exp = s_cur
                    if b + 1 < NBUF:
                        s_cur = emit_buffer(p, qc, b + 1)
                    elif ci + 1 < len(chunks):
                        s_cur = emit_buffer(*chunks[ci + 1], 0)
                    else:
                        s_cur = None
                    if pending:
                        pending.pop(0)(s_exp)
                    # PV for every key tile fully exp'd by now
                    new_kt = (3 * b + nu) // 2
                    for t in range(done_kt, new_kt):
                        for hl in range(2):
                            u = 2 * t + hl
                            psrc, is_i16 = ptiles[u // 3]
                            rhs = psrc[:, (u % 3) * QCH:(u % 3 + 1) * QCH]
                            if is_i16:
                                rhs = rhs.bitcast(bf16)
                            nc.tensor.matmul(
                                ctx[0:7, hl * QCH:(hl + 1) * QCH],
                                lhsT=xo[:, t, :],
                                rhs=rhs,
                                start=(t == 0), stop=(t == NKT - 1),
                                tile_position=(0, 0),
                            )
                    done_kt = new_kt

                # queue this chunk's normalization/output pieces
                pending.append(mk_copy(ctx, ctxsb))
                for c4 in range(NC4):
                    pending.append(mk_group(c4, p, ctxsb, ostage))
                for hl in range(2):
                    pending.append(mk_out(p, qc, hl, ostage))

            # final drain: all group pieces share one fresh ring slot
            # (their ct blocks occupy disjoint columns)
            sx = spool.tile([128, 3 * QCH], f32, name="sx", tag="s")
            while pending:
                pending.pop(0)(sx)

    nc.compile()
    return nc


_NC_CACHE = None


def _get_nc():
    global _NC_CACHE
    if _NC_CACHE is None:
        _NC_CACHE = _build_nc()
    return _NC_CACHE


def _make_in_maps(x, W_query, W_key, W_value):
    in_maps = []
    for c in range(NCORES):
        b = c // 2
        hp = (c % 2) * HPC
        xb = x[b, 0]                                    # [S, 3]

        qstk = np.zeros((128, S), dtype=ml_dtypes.bfloat16)
        kstk = np.zeros((128, S), dtype=ml_dtypes.bfloat16)
        for h in range(HPC):
            Qh = (xb @ W_query[0, hp + h]).T            # [3, S]
            Kh = (xb @ W_key[0, hp + h]).T
            qp = _split3_bf16(Qh)
            kp = _split3_bf16(Kh)
            for t6 in range(6):
                r = 32 * h + 3 * t6
                qstk[r:r + 3] = qp[Q_ORDER[t6]]
                kstk[r:r + 3] = kp[K_ORDER[t6]]

        # xo[p, t, :] = [x_hi(3) | x_lo(3) | 1] at position t*128+p.
        # bf16 hi/lo (~17 mantissa bits combined) keeps the PV weight
        # loads off the FP32-HIGH path, which would disable FWL for
        # every following QK weight load.
        xh = xb.astype(ml_dtypes.bfloat16)
        xl = (xb - xh.astype(np.float32)).astype(ml_dtypes.bfloat16)
        xo = np.concatenate(
            [xh, xl, np.ones((S, 1), ml_dtypes.bfloat16)], axis=1)
        xo = np.ascontiguousarray(
            xo.reshape(NKT, 128, 7).transpose(1, 0, 2))

        # wv7 block for head 2p+hl at partitions 0:7, columns 8p+4hl:
        # rows [Wv; Wv; denom-selector]
        wv7 = np.zeros((128, 16), ml_dtypes.bfloat16)
        for h in range(HPC):
            Wv = W_value[0, hp + h]                     # [3, 3]
            wc = 8 * (h // 2) + 4 * (h % 2)
            wv7[0:3, wc:wc + 3] = Wv
            wv7[3:6, wc:wc + 3] = Wv
            wv7[6, wc + 3] = 1.0

        in_maps.append({
            "qstk": qstk,
            "kstk": kstk,
            "xo": xo,
            "wv7": wv7,
        })
    return in_maps


def kernel(x, W_query, W_key, W_value, _trace=False, _tmpdir=None):
    x = np.asarray(x, dtype=np.float32)
    W_query = np.asarray(W_query, dtype=np.float32)
    W_key = np.asarray(W_key, dtype=np.float32)
    W_value = np.asarray(W_value, dtype=np.float32)

    nc = _get_nc()
    res = run_bass_kernel_spmd(
        nc,
        _make_in_maps(x, W_query, W_key, W_value),
        core_ids=list(range(NCORES)),
        trace=_trace,
        tmpdir=_tmpdir,
    )
    full = np.empty((B, H, S, D), dtype=np.float32)
    for c in range(NCORES):
        b = c // 2
        hp = (c % 2) * HPC
        full[b, hp:hp + HPC] = res.results[c]["out"]
    if _trace:
        kernel._last_results = res
    return (full, full)



# revision 9
# speedup vs baseline: 1.2015x; 1.2015x over previous
"""Multi-head attention kernel for Trainium2 (8 NeuronCores, SPMD).

Problem: x [4,1,2048,3], W_query/W_key/W_value [1,8,3,3] ->
ctx [4,8,2048,3] = softmax((x Wq)(x Wk)^T / sqrt(3)) @ (x Wv), returned
as a (ctx, ctx) tuple matching the reference.

Sharding: 32 (batch, head) blocks over 8 cores -> core c owns batch c//2,
heads 4*(c%2) .. +4. Each core runs an identical Bass program on its slice.

Per-core device program (S=2048, heads processed in 2 pairs x 4 query
chunks of 512; per chunk 32 units of [128 keys, 512 queries]):

  - exp is the element roofline (4*2048*2048 elems). It is SPLIT between
    ACT (true exp at 1 elem/lane/cycle @1.2GHz) and DVE (Schraudolph
    bit-trick: one tensor_scalar computes round(s*C1+C2) into an int16
    tile whose bf16 bitcast IS ~exp(s), <=3.3% per-element, HW-verified
    round-to-nearest; end-to-end rel err ~4e-3 vs the 2e-2 gate).
    Buffers 2,4,6,8,10 of each chunk go to DVE, the rest to ACT, so the
    two engines run the softmax concurrently.

  - PSUM: s pool = 2 x [128, 1536] f32 (banks 0-5, double buffered);
    ctx = one persistent [128, 1024] tile (banks 6-7) per chunk.

  - PE: QK per unit [32,128]x[32,512] row-tiled at the head's 32-row
    group (adjacent units alternate heads -> 2-way concurrency); PV per
    key tile two [128,7]x[128,512] in col group 0 (serialized; full-row
    duty keeps the PE activity monitor warm). PSUM start/stop over the
    16 key tiles does the context reduction for free.

  - Normalization is DECOUPLED from the main loop to kill cross-FIFO
    convoys (ct-matmul -> vector -> PE chains stalled queue heads and
    HAM-cooled the PE): each chunk only copies its raw ctx rows
    [7, 1024] to an SBUF staging slot; a "bundle" (8 tiny transpose+Wv
    matmuls into the just-consumed s slot, 2 broadcast divides, 2 output
    DMAs) runs TWO CHUNKS LATER, when all its deps are long complete.
    Bundles for the last two chunks drain after the loop.
"""

import math

import numpy as np
import ml_dtypes

import concourse.bass as bass
import concourse.bacc as bacc
import concourse.tile as tile
from concourse import mybir
from concourse.bass_utils import run_bass_kernel_spmd

f32 = mybir.dt.float32
f32r = mybir.dt.float32r
bf16 = mybir.dt.bfloat16
i16 = mybir.dt.int16
EXP = mybir.ActivationFunctionType.Exp

B, H, S, D = 4, 8, 2048, 3
NCORES = 8
HPC = H // 2           # heads per core = 4
QCH = 512              # query chunk
NQ = S // QCH          # 4
KT = 128               # key tile
NKT = S // KT          # 16
NC4 = QCH // KT        # 128-query blocks per chunk = 4
NU = 2 * NKT           # units per (pair, chunk) = 32
NBUF = (NU + 2) // 3   # s buffers per chunk = 11 (10x3 + 1x2)
NCH = 2 * NQ           # chunks = 8
SCALE = 1.0 / math.sqrt(D)

# DVE-exp constants: bits16 = round(score*EXPC1 + EXPC2); bitcast bf16.
EXPC1 = SCALE * 128.0 / math.log(2.0)
EXPC2 = 127.0 * 128.0 - 5.568
DVE_BUFS = frozenset((2, 4, 6, 8, 10))

# 3-way bf16 split product terms kept for q.k (drop (2,3),(3,2),(3,3))
Q_ORDER = (0, 0, 1, 0, 2, 1)
K_ORDER = (0, 1, 0, 2, 0, 1)


def _split3_bf16(a: np.ndarray):
    """3-way bf16 split: a ~= a1 + a2 + a3, each bf16."""
    a = np.ascontiguousarray(a, dtype=np.float32)
    a1 = a.astype(ml_dtypes.bfloat16)
    r = a - a1.astype(np.float32)
    a2 = r.astype(ml_dtypes.bfloat16)
    a3 = (r - a2.astype(np.float32)).astype(ml_dtypes.bfloat16)
    return a1, a2, a3


def _build_nc():
    nc = bacc.Bacc("TRN2", target_bir_lowering=False, debug=False,
                   num_devices=NCORES)

    qstk_in = nc.dram_tensor("qstk", [128, S], bf16, kind="ExternalInput").ap()
    kstk_in = nc.dram_tensor("kstk", [128, S], bf16, kind="ExternalInput").ap()
    xo_in = nc.dram_tensor("xo", [128, NKT, 7], bf16, kind="ExternalInput").ap()
    wv7_in = nc.dram_tensor("wv7", [128, 16], bf16, kind="ExternalInput").ap()
    out = nc.dram_tensor("out", [HPC, S, D], f32, kind="ExternalOutput").ap()

    with tile.TileContext(nc) as tc:
        with tc.tile_pool(name="per", bufs=1) as per, \
             tc.tile_pool(name="work", bufs=1) as work, \
             tc.tile_pool(name="spool", bufs=2, space="PSUM") as spool, \
             tc.tile_pool(name="cpool", bufs=1, space="PSUM") as cpool:
            qstk = per.tile([128, S], bf16)
            kstk = per.tile([128, S], bf16)
            xo = per.tile([128, NKT, 7], bf16)
            wv7 = per.tile([128, 16], bf16)
            # raw ctx rows for every chunk live here until its bundle runs
            staging = per.tile([128, NCH, 2 * QCH], bf16)

            # ACT exp-table preload overlaps the input DMAs
            tdum = per.tile([128, 1], f32)
            tdum2 = per.tile([128, 1], f32)
            nc.gpsimd.memset(tdum, 0.0)
            nc.scalar.activation(tdum2, tdum, EXP)

            # pair-0 head rows first so the first QK can start ~1.5us in
            nc.sync.dma_start(out=kstk[0:64, :], in_=kstk_in[0:64, :])
            nc.sync.dma_start(out=qstk[0:64, 0:QCH], in_=qstk_in[0:64, 0:QCH])
            nc.gpsimd.dma_start(out=wv7, in_=wv7_in)
            nc.gpsimd.dma_start(out=xo, in_=xo_in)
            nc.sync.dma_start(out=qstk[0:64, QCH:], in_=qstk_in[0:64, QCH:])
            nc.gpsimd.dma_start(out=kstk[64:128, :], in_=kstk_in[64:128, :])
            nc.sync.dma_start(out=qstk[64:128, :], in_=qstk_in[64:128, :])

            chunks = [(p, qc) for p in range(2) for qc in range(NQ)]
            pending = []  # deferred pieces; each takes the current s buffer

            def emit_buffer(p, qc, b):
                """QK matmuls for s-buffer b of chunk (p, qc)."""
                s = spool.tile([128, 3 * QCH], f32, name=f"s{p}{qc}_{b}",
                               tag="s")
                for u in range(3 * b, min(3 * b + 3, NU)):
                    t, hl = u // 2, u % 2
                    h = 2 * p + hl
                    nc.tensor.matmul(
                        s[:, (u % 3) * QCH:(u % 3 + 1) * QCH],
                        lhsT=kstk[32 * h:32 * h + 32, t * KT:(t + 1) * KT],
                        rhs=qstk[32 * h:32 * h + 32, qc * QCH:(qc + 1) * QCH],
                        start=True, stop=True,
                        tile_position=(32 * h, 0),
                    )
                return s

            def mk_copy(ci, _ctx):
                def go(s_exp):
                    nc.vector.tensor_copy(
                        staging[0:7, ci, 0:2 * QCH], _ctx[0:7, 0:2 * QCH])
                return go

            def mk_bundle(src_ci):
                """Normalize + emit output for chunk src_ci, using the
                current (already exp-consumed) s slot as PE scratch."""
                p, qc = chunks[src_ci]

                def go(s_exp):
                    ost = work.tile([128, NC4, 2, D], f32,
                                    name=f"ost{src_ci}", tag="ost", bufs=2)
                    for c4 in range(NC4):
                        for hl in range(2):
                            base = (1 + hl) * QCH + 16 * c4
                            nc.tensor.matmul(
                                s_exp[:, base:base + 4],
                                lhsT=staging[0:7, src_ci,
                                             hl * QCH + c4 * KT:
                                             hl * QCH + (c4 + 1) * KT],
                                rhs=wv7[0:7, 8 * p + 4 * hl:8 * p + 4 * hl + 4],
                                start=True, stop=True,
                                tile_position=(0, 0),
                            )
                    rec = work.tile([128, 2, NC4], f32,
                                    name=f"rec{src_ci}", tag="rec", bufs=2)
                    for hl in range(2):
                        base = (1 + hl) * QCH
                        nc.vector.reciprocal(
                            rec[:, hl, :],
                            s_exp[:, base + 3:base + 52:16])
                    for hl in range(2):
                        base = (1 + hl) * QCH
                        blk = s_exp[:, base:base + 64].rearrange(
                            "p (c s) -> p c s", s=16)
                        num = blk[:, :, 0:3]
                        sc = rec[:, hl, :].to_broadcast((128, NC4, D))
                        nc.vector.tensor_tensor(
                            out=ost[:, :, hl, :], in0=num, in1=sc,
                            op=mybir.AluOpType.mult)
                    for hl in range(2):
                        h = 2 * p + hl
                        dst = bass.AP(
                            tensor=out.tensor,
                            offset=(h * S * D + qc * QCH * D),
                            ap=[[D, 128], [KT * D, NC4], [1, D]],
                        )
                        nc.sync.dma_start(out=dst, in_=ost[:, :, hl, :])
                return go

            s_cur = emit_buffer(0, 0, 0)
            for ci, (p, qc) in enumerate(chunks):
                ctx = cpool.tile([128, 2 * QCH], f32, name=f"ctx{p}{qc}",
                                 tag="ctx")
                ptiles = {}
                done_kt = 0
                for b in range(NBUF):
                    nu = min(3 * b + 3, NU) - 3 * b
                    fsz = nu * QCH
                    if b in DVE_BUFS:
                        pt = work.tile([128, 3 * QCH], i16,
                                       name=f"pi{p}{qc}_{b}", tag="pi",
                                       bufs=5)
                        ptiles[b] = (pt, True)
                        nc.vector.tensor_scalar(
                            out=pt[:, 0:fsz], in0=s_cur[:, 0:fsz],
                            scalar1=EXPC1, scalar2=EXPC2,
                            op0=mybir.AluOpType.mult,
                            op1=mybir.AluOpType.add)
                    else:
                        pt = work.tile([128, 3 * QCH], bf16,
                                       name=f"p{p}{qc}_{b}", tag="p", bufs=5)
                        ptiles[b] = (pt, False)
                        nc.scalar.activation(pt[:, 0:fsz], s_cur[:, 0:fsz],
                                             EXP, scale=SCALE)
                    s_exp = s_cur
                    if b + 1 < NBUF:
                        s_cur = emit_buffer(p, qc, b + 1)
                    elif ci + 1 < len(chunks):
                        s_cur = emit_buffer(*chunks[ci + 1], 0)
                    else:
                        s_cur = None
                    if pending:
                        pending.pop(0)(s_exp)
                    # PV for every key tile fully exp'd by now
                    new_kt = (3 * b + nu) // 2
                    for t in range(done_kt, new_kt):
                        for hl in range(2):
                            u = 2 * t + hl
                            psrc, is_i16 = ptiles[u // 3]
                            rhs = psrc[:, (u % 3) * QCH:(u % 3 + 1) * QCH]
                            if is_i16:
                                rhs = rhs.bitcast(bf16)
                            nc.tensor.matmul(
                                ctx[0:7, hl * QCH:(hl + 1) * QCH],
                                lhsT=xo[:, t, :],
                                rhs=rhs,
                                start=(t == 0), stop=(t == NKT - 1),
                                tile_position=(0, 0),
                            )
                    done_kt = new_kt

                pending.append(mk_copy(ci, ctx))
                if ci >= 1:
                    pending.append(mk_bundle(ci - 1))

            # drain: remaining pieces (copy of last chunk, bundles 6 and 7)
            pending.append(mk_bundle(NCH - 1))
            sx = spool.tile([128, 3 * QCH], f32, name="sx", tag="s")
            sy = spool.tile([128, 3 * QCH], f32, name="sy", tag="s")
            scratch = [sx, sy]
            i = 0
            while pending:
                pending.pop(0)(scratch[i % 2])
                i += 1

    nc.compile()
    return nc


_NC_CACHE = None


def _get_nc():
    global _NC_CACHE
    if _NC_CACHE is None:
        _NC_CACHE = _build_nc()
    return _NC_CACHE


def _make_in_maps(x, W_query, W_key, W_value):
    in_maps = []
    for c in range(NCORES):
        b = c // 2
        hp = (c % 2) * HPC
        xb = x[b, 0]                                    # [S, 3]

        qstk = np.zeros((128, S), dtype=ml_dtypes.bfloat16)
        kstk = np.zeros((128, S), dtype=ml_dtypes.bfloat16)
        for h in range(HPC):
            Qh = (xb @ W_query[0, hp + h]).T            # [3, S]
            Kh = (xb @ W_key[0, hp + h]).T
            qp = _split3_bf16(Qh)
            kp = _split3_bf16(Kh)
            for t6 in range(6):
                r = 32 * h + 3 * t6
                qstk[r:r + 3] = qp[Q_ORDER[t6]]
                kstk[r:r + 3] = kp[K_ORDER[t6]]

        # xo[p, t, :] = [x_hi(3) | x_lo(3) | 1] at position t*128+p.
        xh = xb.astype(ml_dtypes.bfloat16)
        xl = (xb - xh.astype(np.float32)).astype(ml_dtypes.bfloat16)
        xo = np.concatenate(
            [xh, xl, np.ones((S, 1), ml_dtypes.bfloat16)], axis=1)
        xo = np.ascontiguousarray(
            xo.reshape(NKT, 128, 7).transpose(1, 0, 2))

        # wv7 block for head 2p+hl at partitions 0:7, columns 8p+4hl:
        # rows [Wv; Wv; denom-selector]
        wv7 = np.zeros((128, 16), ml_dtypes.bfloat16)
        for h in range(HPC):
            Wv = W_value[0, hp + h]                     # [3, 3]
            wc = 8 * (h // 2) + 4 * (h % 2)
            wv7[0:3, wc:wc + 3] = Wv
            wv7[3:6, wc:wc + 3] = Wv
            wv7[6, wc + 3] = 1.0

        in_maps.append({
            "qstk": qstk,
            "kstk": kstk,
            "xo": xo,
            "wv7": wv7,
        })
    return in_maps


def kernel(x, W_query, W_key, W_value, _trace=False, _tmpdir=None):
    x = np.asarray(x, dtype=np.float32)
    W_query = np.asarray(W_query, dtype=np.float32)
    W_key = np.asarray(W_key, dtype=np.float32)
    W_value = np.asarray(W_value, dtype=np.float32)

    nc = _get_nc()
    res = run_bass_kernel_spmd(
        nc,
        _make_in_maps(x, W_query, W_key, W_value),
        core_ids=list(range(NCORES)),
        trace=_trace,
        tmpdir=_tmpdir,
    )
    full = np.empty((B, H, S, D), dtype=np.float32)
    for c in range(NCORES):
        b = c // 2
        hp = (c % 2) * HPC
        full[b, hp:hp + HPC] = res.results[c]["out"]
    if _trace:
        kernel._last_results = res
    return (full, full)


# revision 19
# speedup vs baseline: 1.3175x; 1.0965x over previous
"""Multi-head attention kernel for Trainium2 (8 NeuronCores, SPMD).

Problem: x [4,1,2048,3], W_query/W_key/W_value [1,8,3,3] ->
ctx [4,8,2048,3] = softmax((x Wq)(x Wk)^T / sqrt(3)) @ (x Wv), returned
as a (ctx, ctx) tuple matching the reference.

Sharding: 32 (batch, head) blocks over 8 cores -> core c owns batch c//2,
heads 4*(c%2) .. +4. Each core runs an identical Bass program on its slice.

Per-core device program (S=2048, heads processed in 2 pairs x 4 query
chunks of 512; per chunk 32 units of [128 keys, 512 queries]):

  - exp is the element roofline (4*2048*2048 elems). It is SPLIT between
    ACT (true exp at 1 elem/lane/cycle @1.2GHz) and DVE (Schraudolph
    bit-trick: one tensor_scalar computes round(s*C1+C2) into an int16
    tile whose bf16 bitcast IS ~exp(s), <=3.3% per-element, HW-verified
    round-to-nearest; end-to-end rel err ~4e-3 vs the 2e-2 gate).
    Buffers 2,4,6,8,10 of each chunk go to DVE, the rest to ACT, so the
    two engines run the softmax concurrently.

  - PSUM: s pool = 2 x [128, 1536] f32 (banks 0-5, double buffered);
    ctx = one persistent [128, 1024] tile (banks 6-7) per chunk.

  - PE: QK per unit [32,128]x[32,512] row-tiled at the head's 32-row
    group (adjacent units alternate heads -> 2-way concurrency); PV per
    key tile two [128,7]x[128,512] in col group 0 (serialized; full-row
    duty keeps the PE activity monitor warm). PSUM start/stop over the
    16 key tiles does the context reduction for free.

  - Normalization is DECOUPLED from the main loop to kill cross-FIFO
    convoys (ct-matmul -> vector -> PE chains stalled queue heads and
    HAM-cooled the PE): each chunk only copies its raw ctx rows
    [7, 1024] to an SBUF staging slot; a "bundle" (8 tiny transpose+Wv
    matmuls into the just-consumed s slot, 2 broadcast divides, 2 output
    DMAs) runs TWO CHUNKS LATER, when all its deps are long complete.
    Bundles for the last two chunks drain after the loop.
"""

import math

import numpy as np
import ml_dtypes

import concourse.bass as bass
import concourse.bacc as bacc
import concourse.tile as tile
from concourse import mybir
from concourse.bass_utils import run_bass_kernel_spmd

f32 = mybir.dt.float32
f32r = mybir.dt.float32r
bf16 = mybir.dt.bfloat16
i16 = mybir.dt.int16
EXP = mybir.ActivationFunctionType.Exp

B, H, S, D = 4, 8, 2048, 3
NCORES = 8
HPC = H // 2           # heads per core = 4
QCH = 512              # query chunk
NQ = S // QCH          # 4
KT = 128               # key tile
NKT = S // KT          # 16
NC4 = QCH // KT        # 128-query blocks per chunk = 4
NU = 2 * NKT           # units per (pair, chunk) = 32
NBUF = NU // 2         # s buffers per chunk = 16 (2 units each)
NCH = 2 * NQ           # chunks = 8
SCALE = 1.0 / math.sqrt(D)

# DVE-exp constants: bits16 = round(score*EXPC1 + EXPC2); bitcast bf16.
EXPC1 = SCALE * 128.0 / math.log(2.0)
EXPC2 = 127.0 * 128.0 - 5.568
# odd buffers to DVE (consecutive buffers on different engines overlap);
# 9 ACT / 7 DVE balances 1147ns-vs-1192ns tiles + DVE's copy/normalize.
DVE_BUFS = frozenset((1, 3, 5, 7, 9, 11, 13))

# 3-way bf16 split product terms kept for q.k (drop (2,3),(3,2),(3,3))
Q_ORDER = (0, 0, 1, 0, 2, 1)
K_ORDER = (0, 1, 0, 2, 0, 1)


def _split3_bf16(a: np.ndarray):
    """3-way bf16 split: a ~= a1 + a2 + a3, each bf16."""
    a = np.ascontiguousarray(a, dtype=np.float32)
    a1 = a.astype(ml_dtypes.bfloat16)
    r = a - a1.astype(np.float32)
    a2 = r.astype(ml_dtypes.bfloat16)
    a3 = (r - a2.astype(np.float32)).astype(ml_dtypes.bfloat16)
    return a1, a2, a3


def _build_nc():
    nc = bacc.Bacc("TRN2", target_bir_lowering=False, debug=False,
                   num_devices=NCORES)

    qstk_in = nc.dram_tensor("qstk", [128, S], bf16, kind="ExternalInput").ap()
    kstk_in = nc.dram_tensor("kstk", [128, S], bf16, kind="ExternalInput").ap()
    xo_in = nc.dram_tensor("xo", [128, NKT, 7], bf16, kind="ExternalInput").ap()
    wv7_in = nc.dram_tensor("wv7", [128, 16], bf16, kind="ExternalInput").ap()
    out = nc.dram_tensor("out", [HPC, S, D], f32, kind="ExternalOutput").ap()

    with tile.TileContext(nc) as tc:
        with tc.tile_pool(name="per", bufs=1) as per, \
             tc.tile_pool(name="work", bufs=1) as work, \
             tc.tile_pool(name="spool", bufs=3, space="PSUM") as spool, \
             tc.tile_pool(name="cpool", bufs=1, space="PSUM") as cpool:
            qstk = per.tile([128, S], bf16)
            kstk = per.tile([128, S], bf16)
            xo = per.tile([128, NKT, 7], bf16)
            wv7 = per.tile([128, 16], bf16)
            # raw ctx rows for every chunk live here until its bundle runs
            staging = per.tile([128, NCH, 2 * QCH], bf16)

            # ACT exp-table preload overlaps the input DMAs
            tdum = per.tile([128, 1], f32)
            tdum2 = per.tile([128, 1], f32)
            nc.gpsimd.memset(tdum, 0.0)
            nc.scalar.activation(tdum2, tdum, EXP)

            # pair-0 head rows first so the first QK can start ~1.5us in
            nc.sync.dma_start(out=kstk[0:64, :], in_=kstk_in[0:64, :])
            nc.sync.dma_start(out=qstk[0:64, 0:QCH], in_=qstk_in[0:64, 0:QCH])
            nc.gpsimd.dma_start(out=wv7, in_=wv7_in)
            nc.gpsimd.dma_start(out=xo, in_=xo_in)
            nc.sync.dma_start(out=qstk[0:64, QCH:], in_=qstk_in[0:64, QCH:])
            nc.gpsimd.dma_start(out=kstk[64:128, :], in_=kstk_in[64:128, :])
            nc.sync.dma_start(out=qstk[64:128, :], in_=qstk_in[64:128, :])

            chunks = [(p, qc) for p in range(2) for qc in range(NQ)]
            pending = []  # deferred pieces; each takes the current s buffer

            def emit_buffer(p, qc, b):
                """QK matmuls for s-buffer b (2 units) of chunk (p, qc)."""
                s = spool.tile([128, 2 * QCH], f32, name=f"s{p}{qc}_{b}",
                               tag="s")
                for u in range(2 * b, 2 * b + 2):
                    t, hl = u // 2, u % 2
                    h = 2 * p + hl
                    nc.tensor.matmul(
                        s[:, (u % 2) * QCH:(u % 2 + 1) * QCH],
                        lhsT=kstk[32 * h:32 * h + 32, t * KT:(t + 1) * KT],
                        rhs=qstk[32 * h:32 * h + 32, qc * QCH:(qc + 1) * QCH],
                        start=True, stop=True,
                        tile_position=(32 * h, 0),
                    )
                return s

            def mk_copy(ci, _ctx):
                def go(s_exp):
                    nc.vector.tensor_copy(
                        staging[0:7, ci, 0:2 * QCH], _ctx[0:7, 0:2 * QCH])
                return go

            def mk_bundle(src_ci):
                """Normalize + emit output for chunk src_ci, using the
                current (already exp-consumed) s slot as PE scratch."""
                p, qc = chunks[src_ci]

                def go(s_exp):
                    # ct blocks land at cols QCH + 16*(2*c4+hl) of the
                    # (already exp-consumed) scratch slot: [3 outs | denom]
                    ost = work.tile([128, NC4, 2, D], f32,
                                    name=f"ost{src_ci}", tag="ost", bufs=2)
                    for c4 in range(NC4):
                        for hl in range(2):
                            base = QCH + 16 * (2 * c4 + hl)
                            nc.tensor.matmul(
                                s_exp[:, base:base + 4],
                                lhsT=staging[0:7, src_ci,
                                             hl * QCH + c4 * KT:
                                             hl * QCH + (c4 + 1) * KT],
                                rhs=wv7[0:7, 8 * p + 4 * hl:8 * p + 4 * hl + 4],
                                start=True, stop=True,
                                tile_position=(0, 0),
                            )
                    rec = work.tile([128, 2, NC4], f32,
                                    name=f"rec{src_ci}", tag="rec", bufs=2)
                    for hl in range(2):
                        base = QCH + 16 * hl
                        nc.vector.reciprocal(
                            rec[:, hl, :],
                            s_exp[:, base + 3:base + 100:32])
                    for hl in range(2):
                        base = QCH + 16 * hl
                        blk = s_exp[:, base:base + 128].rearrange(
                            "p (c s) -> p c s", s=32)
                        num = blk[:, :, 0:3]
                        sc = rec[:, hl, :].to_broadcast((128, NC4, D))
                        nc.vector.tensor_tensor(
                            out=ost[:, :, hl, :], in0=num, in1=sc,
                            op=mybir.AluOpType.mult)
                    for hl in range(2):
                        h = 2 * p + hl
                        dst = bass.AP(
                            tensor=out.tensor,
                            offset=(h * S * D + qc * QCH * D),
                            ap=[[D, 128], [KT * D, NC4], [1, D]],
                        )
                        nc.sync.dma_start(out=dst, in_=ost[:, :, hl, :])
                return go

            def emit_pv(ctx, ptiles, t):
                for hl in range(2):
                    u = 2 * t + hl
                    psrc, is_i16 = ptiles[u // 2]
                    rhs = psrc[:, (u % 2) * QCH:(u % 2 + 1) * QCH]
                    if is_i16:
                        rhs = rhs.bitcast(bf16)
                    nc.tensor.matmul(
                        ctx[0:7, hl * QCH:(hl + 1) * QCH],
                        lhsT=xo[:, t, :],
                        rhs=rhs,
                        start=(t == 0), stop=(t == NKT - 1),
                        tile_position=(0, 0),
                    )

            s_cur = emit_buffer(0, 0, 0)
            s_exp = None
            for ci, (p, qc) in enumerate(chunks):
                ctx = cpool.tile([128, 2 * QCH], f32, name=f"ctx{p}{qc}",
                                 tag="ctx")
                ptiles = {}
                for b in range(NBUF):
                    fsz = 2 * QCH
                    if b in DVE_BUFS:
                        pt = work.tile([128, 2 * QCH], i16,
                                       name=f"pi{p}{qc}_{b}", tag="pi",
                                       bufs=5)
                        ptiles[b] = (pt, True)
                        nc.vector.tensor_scalar(
                            out=pt[:, 0:fsz], in0=s_cur[:, 0:fsz],
                            scalar1=EXPC1, scalar2=EXPC2,
                            op0=mybir.AluOpType.mult,
                            op1=mybir.AluOpType.add)
                    else:
                        pt = work.tile([128, 2 * QCH], bf16,
                                       name=f"p{p}{qc}_{b}", tag="p", bufs=5)
                        ptiles[b] = (pt, False)
                        nc.scalar.activation(pt[:, 0:fsz], s_cur[:, 0:fsz],
                                             EXP, scale=SCALE)
                    s_prev = s_exp  # slot of buffer b-1: exp long done
                    s_exp = s_cur
                    if b + 1 < NBUF:
                        s_cur = emit_buffer(p, qc, b + 1)
                    elif ci + 1 < len(chunks):
                        s_cur = emit_buffer(*chunks[ci + 1], 0)
                    else:
                        s_cur = None
                    # PV trails by one buffer so it never head-blocks the
                    # PE queue waiting on this buffer's exp
                    if b > 0:
                        emit_pv(ctx, ptiles, b - 1)
                    # normalize pieces: fire on ACT buffers (vector queue
                    # free) with the previous slot (exp complete) as scratch
                    if pending and b % 2 == 0 and s_prev is not None:
                        pending.pop(0)(s_prev)

                emit_pv(ctx, ptiles, NBUF - 1)
                pending.append(mk_copy(ci, ctx))
                if ci >= 1:
                    pending.append(mk_bundle(ci - 1))

            # drain: remaining pieces (copy of last chunk, bundles 6 and 7)
            pending.append(mk_bundle(NCH - 1))
            sx = spool.tile([128, 2 * QCH], f32, name="sx", tag="s")
            sy = spool.tile([128, 2 * QCH], f32, name="sy", tag="s")
            scratch = [sx, sy]
            i = 0
            while pending:
                pending.pop(0)(scratch[i % 2])
                i += 1

    nc.compile()
    return nc


_NC_CACHE = None


def _get_nc():
    global _NC_CACHE
    if _NC_CACHE is None:
        _NC_CACHE = _build_nc()
    return _NC_CACHE


def _make_in_maps(x, W_query, W_key, W_value):
    in_maps = []
    for c in range(NCORES):
        b = c // 2
        hp = (c % 2) * HPC
        xb = x[b, 0]                                    # [S, 3]

        qstk = np.zeros((128, S), dtype=ml_dtypes.bfloat16)
        kstk = np.zeros((128, S), dtype=ml_dtypes.bfloat16)
        for h in range(HPC):
            Qh = (xb @ W_query[0, hp + h]).T            # [3, S]
            Kh = (xb @ W_key[0, hp + h]).T
            qp = _split3_bf16(Qh)
            kp = _split3_bf16(Kh)
            for t6 in range(6):
                r = 32 * h + 3 * t6
                qstk[r:r + 3] = qp[Q_ORDER[t6]]
                kstk[r:r + 3] = kp[K_ORDER[t6]]

        # xo[p, t, :] = [x_hi(3) | x_lo(3) | 1] at position t*128+p.
        xh = xb.astype(ml_dtypes.bfloat16)
        xl = (xb - xh.astype(np.float32)).astype(ml_dtypes.bfloat16)
        xo = np.concatenate(
            [xh, xl, np.ones((S, 1), ml_dtypes.bfloat16)], axis=1)
        xo = np.ascontiguousarray(
            xo.reshape(NKT, 128, 7).transpose(1, 0, 2))

        # wv7 block for head 2p+hl at partitions 0:7, columns 8p+4hl:
        # rows [Wv; Wv; denom-selector]
        wv7 = np.zeros((128, 16), ml_dtypes.bfloat16)
        for h in range(HPC):
            Wv = W_value[0, hp + h]                     # [3, 3]
            wc = 8 * (h // 2) + 4 * (h % 2)
            wv7[0:3, wc:wc + 3] = Wv
            wv7[3:6, wc:wc + 3] = Wv
            wv7[6, wc + 3] = 1.0

        in_maps.append({
            "qstk": qstk,
            "kstk": kstk,
            "xo": xo,
            "wv7": wv7,
        })
    return in_maps


def kernel(x, W_query, W_key, W_value, _trace=False, _tmpdir=None):
    x = np.asarray(x, dtype=np.float32)
    W_query = np.asarray(W_query, dtype=np.float32)
    W_key = np.asarray(W_key, dtype=np.float32)
    W_value = np.asarray(W_value, dtype=np.float32)

    nc = _get_nc()
    res = run_bass_kernel_spmd(
        nc,
        _make_in_maps(x, W_query, W_key, W_value),
        core_ids=list(range(NCORES)),
        trace=_trace,
        tmpdir=_tmpdir,
    )
    full = np.empty((B, H, S, D), dtype=np.float32)
    for c in range(NCORES):
        b = c // 2
        hp = (c % 2) * HPC
        full[b, hp:hp + HPC] = res.results[c]["out"]
    if _trace:
        kernel._last_results = res
    return (full, full)


# revision 26
# speedup vs baseline: 1.3927x; 1.0571x over previous
"""Multi-head attention kernel for Trainium2 (8 NeuronCores, SPMD).

Problem: x [4,1,2048,3], W_query/W_key/W_value [1,8,3,3] ->
ctx [4,8,2048,3] = softmax((x Wq)(x Wk)^T / sqrt(3)) @ (x Wv), returned
as a (ctx, ctx) tuple matching the reference.

Sharding: 32 (batch, head) blocks over 8 cores -> core c owns batch c//2,
heads 4*(c%2) .. +4. Each core runs an identical Bass program on its slice.

Per-core device program (S=2048, heads processed in 2 pairs x 4 query
chunks of 512; per chunk 32 units of [128 keys, 512 queries]):

  - exp is the element roofline (4*2048*2048 elems). It is SPLIT between
    ACT (true exp at 1 elem/lane/cycle @1.2GHz) and DVE (Schraudolph
    bit-trick: one tensor_scalar computes round(s*C1+C2) into an int16
    tile whose bf16 bitcast IS ~exp(s), <=3.3% per-element, HW-verified
    round-to-nearest; end-to-end rel err ~4e-3 vs the 2e-2 gate).
    Buffers 2,4,6,8,10 of each chunk go to DVE, the rest to ACT, so the
    two engines run the softmax concurrently.

  - PSUM: s pool = 2 x [128, 1536] f32 (banks 0-5, double buffered);
    ctx = one persistent [128, 1024] tile (banks 6-7) per chunk.

  - PE: QK per unit [32,128]x[32,512] row-tiled at the head's 32-row
    group (adjacent units alternate heads -> 2-way concurrency); PV per
    key tile two [128,7]x[128,512] in col group 0 (serialized; full-row
    duty keeps the PE activity monitor warm). PSUM start/stop over the
    16 key tiles does the context reduction for free.

  - Normalization is DECOUPLED from the main loop to kill cross-FIFO
    convoys (ct-matmul -> vector -> PE chains stalled queue heads and
    HAM-cooled the PE): each chunk only copies its raw ctx rows
    [7, 1024] to an SBUF staging slot; a "bundle" (8 tiny transpose+Wv
    matmuls into the just-consumed s slot, 2 broadcast divides, 2 output
    DMAs) runs TWO CHUNKS LATER, when all its deps are long complete.
    Bundles for the last two chunks drain after the loop.
"""

import math

import numpy as np
import ml_dtypes

import concourse.bass as bass
import concourse.bacc as bacc
import concourse.tile as tile
from concourse import mybir
from concourse.bass_utils import run_bass_kernel_spmd

f32 = mybir.dt.float32
f32r = mybir.dt.float32r
bf16 = mybir.dt.bfloat16
i16 = mybir.dt.int16
EXP = mybir.ActivationFunctionType.Exp

B, H, S, D = 4, 8, 2048, 3
NCORES = 8
HPC = H // 2           # heads per core = 4
QCH = 512              # query chunk
NQ = S // QCH          # 4
KT = 128               # key tile
NKT = S // KT          # 16
NC4 = QCH // KT        # 128-query blocks per chunk = 4
NU = 2 * NKT           # units per (pair, chunk) = 32
NBUF = NU // 2         # s buffers per chunk = 16 (2 units each)
NCH = 2 * NQ           # chunks = 8
SCALE = 1.0 / math.sqrt(D)

# DVE-exp constants: bits16 = round(score*EXPC1 + EXPC2); bitcast bf16.
EXPC1 = SCALE * 128.0 / math.log(2.0)
EXPC2 = 127.0 * 128.0 - 5.568
# even buffers to DVE, odd to ACT: consecutive buffers overlap engines and
# ACT opens each chunk with the ctx->staging copy instead of an exp.
DVE_BUFS = frozenset(range(0, NBUF, 2))

# 3-way bf16 split product terms kept for q.k (drop (2,3),(3,2),(3,3))
Q_ORDER = (0, 0, 1, 0, 2, 1)
K_ORDER = (0, 1, 0, 2, 0, 1)


def _split3_bf16(a: np.ndarray):
    """3-way bf16 split: a ~= a1 + a2 + a3, each bf16."""
    a = np.ascontiguousarray(a, dtype=np.float32)
    a1 = a.astype(ml_dtypes.bfloat16)
    r = a - a1.astype(np.float32)
    a2 = r.astype(ml_dtypes.bfloat16)
    a3 = (r - a2.astype(np.float32)).astype(ml_dtypes.bfloat16)
    return a1, a2, a3


def _build_nc():
    nc = bacc.Bacc("TRN2", target_bir_lowering=False, debug=False,
                   num_devices=NCORES)

    qstk_in = [nc.dram_tensor(f"qstk{p}", [128, S], bf16,
                              kind="ExternalInput").ap() for p in range(2)]
    kstk_in = [nc.dram_tensor(f"kstk{p}", [128, S], bf16,
                              kind="ExternalInput").ap() for p in range(2)]
    xo_in = nc.dram_tensor("xo", [128, NKT, 7], bf16, kind="ExternalInput").ap()
    wv7_in = nc.dram_tensor("wv7", [128, 16], bf16, kind="ExternalInput").ap()
    out = nc.dram_tensor("out", [HPC, S, D], f32, kind="ExternalOutput").ap()

    with tile.TileContext(nc) as tc:
        with tc.tile_pool(name="per", bufs=1) as per, \
             tc.tile_pool(name="work", bufs=1) as work, \
             tc.tile_pool(name="spool", bufs=3, space="PSUM") as spool, \
             tc.tile_pool(name="cpool", bufs=1, space="PSUM") as cpool:
            qstk = [per.tile([128, S], bf16, name=f"qs{p}") for p in range(2)]
            kstk = [per.tile([128, S], bf16, name=f"ks{p}") for p in range(2)]
            xo = per.tile([128, NKT, 7], bf16)
            wv7 = per.tile([128, 16], bf16)
            # raw ctx rows for every chunk live here until its bundle runs
            staging = per.tile([128, NCH, 2 * QCH], bf16)

            # ACT exp-table preload overlaps the input DMAs
            tdum = per.tile([128, 1], f32)
            tdum2 = per.tile([128, 1], f32)
            nc.gpsimd.memset(tdum, 0.0)
            nc.scalar.activation(tdum2, tdum, EXP)

            # HAM warmup: ~4us of full-row dummy matmuls on a memset tile
            # flips the PE clock gate to 8/8 before the real work arrives
            # (a cold start costs ~13us: first ~26us ran at 1.2 GHz).
            warm = per.tile([128, 5 * KT], bf16)
            dummy = cpool.tile([128, 2 * QCH], f32, name="dummyctx",
                               tag="ctx")
            nc.gpsimd.memset(warm, 0.0)
            for i in range(10):
                nc.tensor.matmul(
                    dummy[:, 0:QCH], lhsT=warm[:, 0:KT], rhs=warm[:, KT:],
                    start=True, stop=True, tile_position=(0, 0))

            # pair-0 rows first so the first QKs can start early
            nc.sync.dma_start(out=kstk[0][0:64, :], in_=kstk_in[0][0:64, :])
            nc.sync.dma_start(out=qstk[0][0:64, 0:QCH],
                              in_=qstk_in[0][0:64, 0:QCH])
            nc.gpsimd.dma_start(out=kstk[0][64:128, :],
                                in_=kstk_in[0][64:128, :])
            nc.gpsimd.dma_start(out=wv7, in_=wv7_in)
            nc.sync.dma_start(out=qstk[0][64:128, 0:QCH],
                              in_=qstk_in[0][64:128, 0:QCH])
            nc.gpsimd.dma_start(out=xo, in_=xo_in)
            nc.sync.dma_start(out=qstk[0][0:64, QCH:],
                              in_=qstk_in[0][0:64, QCH:])
            nc.gpsimd.dma_start(out=qstk[0][64:128, QCH:],
                                in_=qstk_in[0][64:128, QCH:])
            nc.sync.dma_start(out=kstk[1], in_=kstk_in[1])
            nc.gpsimd.dma_start(out=qstk[1], in_=qstk_in[1])

            chunks = [(p, qc) for p in range(2) for qc in range(NQ)]
            pending = []  # deferred pieces; each takes the current s buffer

            def emit_buffer(p, qc, b):
                """QK matmuls for s-buffer b (2 units = 1 key tile) of chunk
                (p, qc). Row-group layout: even key tiles use groups {0,1},
                odd {2,3}, so the next buffer's LDWEIGHTS prefetches into
                idle groups while this buffer's matmuls stream."""
                s = spool.tile([128, 2 * QCH], f32, name=f"s{p}{qc}_{b}",
                               tag="s")
                t = b
                for hl in range(2):
                    base = 64 * (t % 2) + 32 * hl
                    nc.tensor.matmul(
                        s[:, hl * QCH:(hl + 1) * QCH],
                        lhsT=kstk[p][base:base + 32, t * KT:(t + 1) * KT],
                        rhs=qstk[p][base:base + 32,
                                    qc * QCH:(qc + 1) * QCH],
                        start=True, stop=True,
                        tile_position=(base, 0),
                    )
                return s

            def mk_copy(ci, _ctx):
                # on ACT: it opens each chunk (DVE owns buffer 0's exp), so
                # the next chunk's first PV never waits on this copy
                def go(s_exp):
                    nc.scalar.copy(
                        staging[0:7, ci, 0:2 * QCH], _ctx[0:7, 0:2 * QCH])
                return go

            def mk_bundle(src_ci):
                """Normalize + emit output for chunk src_ci, using the
                current (already exp-consumed) s slot as PE scratch."""
                p, qc = chunks[src_ci]

                def go(s_exp):
                    # ct blocks land at cols QCH + 16*(2*c4+hl) of the
                    # (already exp-consumed) scratch slot: [3 outs | denom]
                    ost = work.tile([128, NC4, 2, D], f32,
                                    name=f"ost{src_ci}", tag="ost", bufs=2)
                    for c4 in range(NC4):
                        for hl in range(2):
                            base = QCH + 16 * (2 * c4 + hl)
                            nc.tensor.matmul(
                                s_exp[:, base:base + 4],
                                lhsT=staging[0:7, src_ci,
                                             hl * QCH + c4 * KT:
                                             hl * QCH + (c4 + 1) * KT],
                                rhs=wv7[0:7, 8 * p + 4 * hl:8 * p + 4 * hl + 4],
                                start=True, stop=True,
                                tile_position=(0, 0),
                            )
                    rec = work.tile([128, 2, NC4], f32,
                                    name=f"rec{src_ci}", tag="rec", bufs=2)
                    for hl in range(2):
                        base = QCH + 16 * hl
                        nc.vector.reciprocal(
                            rec[:, hl, :],
                            s_exp[:, base + 3:base + 100:32])
                    for hl in range(2):
                        base = QCH + 16 * hl
                        blk = s_exp[:, base:base + 128].rearrange(
                            "p (c s) -> p c s", s=32)
                        num = blk[:, :, 0:3]
                        sc = rec[:, hl, :].to_broadcast((128, NC4, D))
                        nc.vector.tensor_tensor(
                            out=ost[:, :, hl, :], in0=num, in1=sc,
                            op=mybir.AluOpType.mult)
                    for hl in range(2):
                        h = 2 * p + hl
                        dst = bass.AP(
                            tensor=out.tensor,
                            offset=(h * S * D + qc * QCH * D),
                            ap=[[D, 128], [KT * D, NC4], [1, D]],
                        )
                        nc.sync.dma_start(out=dst, in_=ost[:, :, hl, :])
                return go

            def emit_pv(ctx, ptiles, t):
                for hl in range(2):
                    u = 2 * t + hl
                    psrc, is_i16 = ptiles[u // 2]
                    rhs = psrc[:, (u % 2) * QCH:(u % 2 + 1) * QCH]
                    if is_i16:
                        rhs = rhs.bitcast(bf16)
                    nc.tensor.matmul(
                        ctx[0:7, hl * QCH:(hl + 1) * QCH],
                        lhsT=xo[:, t, :],
                        rhs=rhs,
                        start=(t == 0), stop=(t == NKT - 1),
                        tile_position=(0, 0),
                    )

            s_cur = emit_buffer(0, 0, 0)
            s_exp = None
            for ci, (p, qc) in enumerate(chunks):
                ctx = cpool.tile([128, 2 * QCH], f32, name=f"ctx{p}{qc}",
                                 tag="ctx")
                ptiles = {}
                for b in range(NBUF):
                    fsz = 2 * QCH
                    if b in DVE_BUFS:
                        pt = work.tile([128, 2 * QCH], i16,
                                       name=f"pi{p}{qc}_{b}", tag="pi",
                                       bufs=5)
                        ptiles[b] = (pt, True)
                        nc.vector.tensor_scalar(
                            out=pt[:, 0:fsz], in0=s_cur[:, 0:fsz],
                            scalar1=EXPC1, scalar2=EXPC2,
                            op0=mybir.AluOpType.mult,
                            op1=mybir.AluOpType.add)
                    else:
                        pt = work.tile([128, 2 * QCH], bf16,
                                       name=f"p{p}{qc}_{b}", tag="p", bufs=5)
                        ptiles[b] = (pt, False)
                        nc.scalar.activation(pt[:, 0:fsz], s_cur[:, 0:fsz],
                                             EXP, scale=SCALE)
                    s_prev = s_exp  # slot of buffer b-1: exp long done
                    s_exp = s_cur
                    if b + 1 < NBUF:
                        s_cur = emit_buffer(p, qc, b + 1)
                    elif ci + 1 < len(chunks):
                        s_cur = emit_buffer(*chunks[ci + 1], 0)
                    else:
                        s_cur = None
                    # PV trails by one buffer so it never head-blocks the
                    # PE queue waiting on this buffer's exp
                    if b > 0:
                        emit_pv(ctx, ptiles, b - 1)
                    # normalize pieces: fire on ACT buffers (vector queue
                    # free) with the previous slot (exp complete) as scratch
                    if pending and b % 2 == 0 and s_prev is not None:
                        pending.pop(0)(s_prev)

                emit_pv(ctx, ptiles, NBUF - 1)
                pending.append(mk_copy(ci, ctx))
                if ci >= 1:
                    pending.append(mk_bundle(ci - 1))

            # drain: remaining pieces (copy of last chunk, bundles 6 and 7)
            pending.append(mk_bundle(NCH - 1))
            sx = spool.tile([128, 2 * QCH], f32, name="sx", tag="s")
            sy = spool.tile([128, 2 * QCH], f32, name="sy", tag="s")
            scratch = [sx, sy]
            i = 0
            while pending:
                pending.pop(0)(scratch[i % 2])
                i += 1

    nc.compile()
    return nc


_NC_CACHE = None


def _get_nc():
    global _NC_CACHE
    if _NC_CACHE is None:
        _NC_CACHE = _build_nc()
    return _NC_CACHE


def _make_in_maps(x, W_query, W_key, W_value):
    in_maps = []
    for c in range(NCORES):
        b = c // 2
        hp = (c % 2) * HPC
        xb = x[b, 0]                                    # [S, 3]

        # per-pair stacks; each head's 18 split rows are duplicated into
        # two 32-row groups (64*grp + 32*hl) so even/odd key tiles hit
        # disjoint PE row groups (LDWEIGHTS prefetch overlap)
        qstk = [np.zeros((128, S), dtype=ml_dtypes.bfloat16)
                for _ in range(2)]
        kstk = [np.zeros((128, S), dtype=ml_dtypes.bfloat16)
                for _ in range(2)]
        for h in range(HPC):
            pq, hl = h // 2, h % 2
            Qh = (xb @ W_query[0, hp + h]).T            # [3, S]
            Kh = (xb @ W_key[0, hp + h]).T
            qp = _split3_bf16(Qh)
            kp = _split3_bf16(Kh)
            for t6 in range(6):
                for grp in range(2):
                    r = 64 * grp + 32 * hl + 3 * t6
                    qstk[pq][r:r + 3] = qp[Q_ORDER[t6]]
                    kstk[pq][r:r + 3] = kp[K_ORDER[t6]]

        # xo[p, t, :] = [x_hi(3) | x_lo(3) | 1] at position t*128+p.
        xh = xb.astype(ml_dtypes.bfloat16)
        xl = (xb - xh.astype(np.float32)).astype(ml_dtypes.bfloat16)
        xo = np.concatenate(
            [xh, xl, np.ones((S, 1), ml_dtypes.bfloat16)], axis=1)
        xo = np.ascontiguousarray(
            xo.reshape(NKT, 128, 7).transpose(1, 0, 2))

        # wv7 block for head 2p+hl at partitions 0:7, columns 8p+4hl:
        # rows [Wv; Wv; denom-selector]
        wv7 = np.zeros((128, 16), ml_dtypes.bfloat16)
        for h in range(HPC):
            Wv = W_value[0, hp + h]                     # [3, 3]
            wc = 8 * (h // 2) + 4 * (h % 2)
            wv7[0:3, wc:wc + 3] = Wv
            wv7[3:6, wc:wc + 3] = Wv
            wv7[6, wc + 3] = 1.0

        in_maps.append({
            "qstk0": qstk[0],
            "qstk1": qstk[1],
            "kstk0": kstk[0],
            "kstk1": kstk[1],
            "xo": xo,
            "wv7": wv7,
        })
    return in_maps


def kernel(x, W_query, W_key, W_value, _trace=False, _tmpdir=None):
    x = np.asarray(x, dtype=np.float32)
    W_query = np.asarray(W_query, dtype=np.float32)
    W_key = np.asarray(W_key, dtype=np.float32)
    W_value = np.asarray(W_value, dtype=np.float32)

    nc = _get_nc()
    res = run_bass_kernel_spmd(
        nc,
        _make_in_maps(x, W_query, W_key, W_value),
        core_ids=list(range(NCORES)),
        trace=_trace,
        tmpdir=_tmpdir,
    )
    full = np.empty((B, H, S, D), dtype=np.float32)
    for c in range(NCORES):
        b = c // 2
        hp = (c % 2) * HPC
        full[b, hp:hp + HPC] = res.results[c]["out"]
    if _trace:
        kernel._last_results = res
    return (full, full)
